# revision 72
# baseline (speedup 1.0000x reference)
"""Trainium2 Bass kernel for CtrlPointHungarianMatcher cost matrix.

Computes C[b,q, b'*NGT+g] = class_cost[b,q] + L1_cdist + blockdiag(text_KL).

Sharding: data-parallel over batch; core c handles images (2c, 2c+1) =
200 queries x all 512 targets.

Design notes (final):
- The L1 cdist is a rank-14 bilinear form: |x-y| on [0,1]^2 is
  approximated by sum_r f_r(x) g_r(y) (SVD of the kernel on a 256-pt
  grid; max abs err ~0.11 on 50-dim sums, ~1.3% of the smallest C).
  Host evaluates the factor tables at input coords (input encoding,
  like the baseline's char histogram); the device contracts the
  [701-row] factors into 4 [128t, 200q] PSUM tiles.  Ranks 0-3 ride
  in f16 (normal matmuls); ranks 4-13 in float8_e4m3 run PAIRWISE in
  DoubleRow perf mode (2 contraction chunks per instruction).  All
  sides are scaled x4 (uniform x16 product scale, divided out on the
  host), which keeps every value in the fp8 normal range.
- pred_text_logits are shipped as float8_e3m4 (error on log-probs
  <0.05, ~0.5% of C) — halves the dominant input DMA vs bf16.
- All activations are Exp/Ln/Copy; a combined exp+ln table
  (natural_log_exp_and_others) is pre-loaded once at program start so
  the ACT engine never reloads tables (an earlier trace showed
  6 x 1.28us of exp<->ln table thrash).  Sigmoid for the focal cost
  is exp+reciprocal to stay inside that one table.
- The focal class cost joins the cdist PSUM accumulation as a rank-1
  broadcast matmul (ones[1,128] x class-row[1,200]), computed early so
  it never stalls the accumulation's stop.
- Inputs stream on three parallel DMA queues (SP: consts+ptl slices,
  ACT: f16 factors, Pool: fp8 factors); per-queue throughput is only
  ~160GB/s so queue parallelism matters.  ptl lands in 3 slices and
  is exp/softmax'd in 5 slices of 8 groups so the softmax pipeline
  overlaps the stream; each [128t, 200q] output block DMAs out as
  soon as its PSUM->SBUF copy lands.
Host un-permutes [128,4*200] -> [200,512]/16 per core and adds the
text block diagonally (core-dependent column offset; SPMD program is
shared).
"""

import sys

sys.path.insert(0, "/opt/trn_rl_repo")

from contextlib import ExitStack

import ml_dtypes
import numpy as np

from concourse import bacc, bass, mybir, tile
from concourse import bass_utils

BF16 = mybir.dt.bfloat16
F32 = mybir.dt.float32
F16 = mybir.dt.float16
FP8 = mybir.dt.float8e3       # e3m4: pred-text logits (best <1 precision)
FP8W = mybir.dt.float8e4      # e4m3: cdist tail-rank factors (DoubleRow-able)
AF = mybir.ActivationFunctionType
OP = mybir.AluOpType

NPBF16 = ml_dtypes.bfloat16
NPFP8 = ml_dtypes.float8_e3m4
NPFP8W = ml_dtypes.float8_e4m3fn

BS, NQ, NPTS, VOC, MAXLEN, NGT, EDIM = 16, 100, 25, 96, 25, 32, 300
NCORES = 8
NI = BS // NCORES          # images per core = 2
T = BS * NGT               # 512 targets
D = NPTS * 2               # 50 coord dims
NQC = NI * NQ              # 200 queries per core
INV_SQRT_E = float(1.0 / np.sqrt(np.float32(EDIM)))

# rank-R bilinear factorization of |x-y|; ranks [0,NH) f16, [NH,R) fp8e4.
# fp8 chunks run pairwise in DoubleRow mode (2 contraction rows/PE row),
# so the fp8 chunk count is padded to even.
RNK, NH = 14, 4
GRID = 256
N16 = D * NH               # 200 f16 rows
N8 = D * (RNK - NH)        # 700 fp8 rows
CH16 = [min(128, N16 - 128 * c) for c in range((N16 + 127) // 128)]  # [128, 72]
NCH8 = -2 * ((N8 + 127) // 128 // -2)                                # 6 (even)
CH8 = [min(128, max(0, N8 - 128 * c)) for c in range(NCH8)]
SCL = 4.0                  # per-side scale; product scale 16, host divides

# ptl DMA'd in 3 slices; exp/sums processed in 5 slices of 8 groups
# (40 softmax groups of 97 cols)
GDMA = [(0, 16), (16, 32), (32, 40)]
GQ = [(0, 8), (8, 16), (16, 24), (24, 32), (32, 38), (38, 40)]



_CACHE = {}


def _basis():
    x = (np.arange(GRID, dtype=np.float64) + 0.5) / GRID
    A = np.abs(x[:, None] - x[None, :])
    U, s, Vt = np.linalg.svd(A)
    Fb = (U[:, :RNK] * np.sqrt(s[:RNK])).astype(np.float32)
    Gb = (Vt[:RNK].T * np.sqrt(s[:RNK])).astype(np.float32)
    return Fb, Gb


def _ev(P, pts):
    """Linear interp of basis table P [GRID, R] at pts [...] -> [..., R]."""
    idx = np.clip(pts.astype(np.float64) * GRID - 0.5, 0, GRID - 1 - 1e-9)
    i0 = np.floor(idx).astype(np.int32)
    fr = (idx - i0)[..., None].astype(np.float32)
    i1 = np.minimum(i0 + 1, GRID - 1)
    return P[i0] * (1 - fr) + P[i1] * fr


def _chunk(rows, nch, width, dtype):
    """[n_used, width] -> [128, nch*width] chunk-major layout."""
    arr = np.zeros((nch * 128, width), dtype)
    arr[: rows.shape[0]] = rows
    return np.ascontiguousarray(
        arr.reshape(nch, 128, width).transpose(1, 0, 2).reshape(128, nch * width))


def _factor_blocks(coords, P, width):
    """coords [width, 50] -> (f16 block [128, len(CH16)*width],
    fp8e4 block [128, len(CH8)*width]), scaled by SCL."""
    fv = _ev(P, coords) * SCL                       # [width, 50, R]
    hi = fv[..., :NH].transpose(1, 2, 0).reshape(N16, width)
    lo = fv[..., NH:].transpose(1, 2, 0).reshape(N8, width)
    return (_chunk(hi.astype(np.float16), len(CH16), width, np.float16),
            _chunk(lo.astype(NPFP8W), len(CH8), width, NPFP8W))


def _build_program():
    nc = bacc.Bacc("TRN2", debug=False, num_devices=NCORES)

    t_ptl = nc.dram_tensor("ptl", [125, 40 * 97], FP8, kind="ExternalInput")
    t_f16b = nc.dram_tensor("f16b", [128, len(CH16) * (T + NQC)], F16,
                            kind="ExternalInput")
    t_fp8b = nc.dram_tensor("fp8b", [128, len(CH8) * (T + NQC)], FP8W,
                            kind="ExternalInput")
    t_b16 = nc.dram_tensor("b16c", [128, 635], BF16, kind="ExternalInput")

    t_out = nc.dram_tensor("outC", [128, 4 * NQC], F16, kind="ExternalOutput")
    t_txt = nc.dram_tensor("outT", [2 * NGT, NQ], F16, kind="ExternalOutput")

    with tile.TileContext(nc) as tc:
        with ExitStack() as ctx:
            _body(ctx, tc, t_ptl, t_f16b, t_fp8b, t_b16, t_out, t_txt)
    nc.compile()
    return nc


def _act_table_id(arch):
    from concourse.hw_specs import get_activation_tables
    tables = get_activation_tables(arch)
    for i, (name, funcs) in enumerate(tables.items()):
        if name == "natural_log_exp_and_others":
            return i
    return None


def _body(ctx, tc, t_ptl, t_f16b, t_fp8b, t_b16, t_out, t_txt):
    nc = tc.nc

    const = ctx.enter_context(tc.tile_pool(name="const", bufs=1))
    work = ctx.enter_context(tc.tile_pool(name="work", bufs=1))
    psum = ctx.enter_context(tc.tile_pool(name="psum", bufs=1, space="PSUM"))

    # pre-load the combined exp+ln table so no reloads happen mid-kernel
    tid = _act_table_id(nc.m.arch)
    if tid is not None:
        ld = mybir.InstLoadActFuncSet(
            name=nc.get_next_instruction_name(), act_func_set_id=tid)
        nc.scalar.add_instruction(ld)



    # ---------------- input DMAs: three hardware queues in parallel ------
    # (each queue streams ~100-160GB/s; queues run concurrently)
    bb = const.tile([128, 635], BF16, tag="bb")
    nc.sync.dma_start(bb[:], t_b16.ap())
    ptl = work.tile([125, 40 * 97], FP8, tag="ptl")
    for g0, g1 in GDMA:
        nc.sync.dma_start(ptl[:, 97 * g0 : 97 * g1],
                          t_ptl.ap()[:, 97 * g0 : 97 * g1])
    f16b = const.tile([128, len(CH16) * (T + NQC)], F16, tag="f16b")
    nc.scalar.dma_start(f16b[:], t_f16b.ap())
    fp8b = const.tile([128, len(CH8) * (T + NQC)], FP8W, tag="fp8b")
    nc.gpsimd.dma_start(fp8b[:], t_fp8b.ap())
    ct16 = f16b[:, : len(CH16) * T]
    cq16 = f16b[:, len(CH16) * T :]
    ct8 = fp8b[:, : len(CH8) * T]
    cq8 = fp8b[:, len(CH8) * T :]

    cent = bb[:, 0:288]
    ident = bb[:, 288:388]
    histT = bb[:96, 388:452]
    pls = bb[:100, 452:502]
    sel04 = bb[:125, 502:507]
    onesr = bb[0:1, 507:635]

    # ---------------- target text distributions -------------------------
    G = psum.tile([VOC, VOC], F32, tag="mmA")
    for kk in range(3):
        cch = cent[:, 96 * kk : 96 * (kk + 1)]
        nc.tensor.matmul(G[:], cch, cch, start=(kk == 0), stop=(kk == 2))


    gmaxn = work.tile([VOC, 1], F32, tag="gmaxn")
    nc.vector.tensor_reduce(gmaxn[:], G[:], axis=mybir.AxisListType.X,
                            op=OP.max, negate=True)
    gbias = work.tile([VOC, 1], F32, tag="gbias")
    nc.vector.tensor_scalar(gbias[:], gmaxn[:], INV_SQRT_E, None, op0=OP.mult)
    S0 = work.tile([VOC, VOC], F32, tag="S0")
    ssum = work.tile([VOC, 1], F32, tag="ssum")
    nc.scalar.activation(S0[:], G[:], AF.Exp, bias=gbias[:], scale=INV_SQRT_E,
                         accum_out=ssum[:])
    srec = work.tile([VOC, 1], F32, tag="srec")
    nc.vector.reciprocal(srec[:], ssum[:])
    Ssb = work.tile([VOC, VOC], BF16, tag="Ssb")
    nc.vector.tensor_scalar(Ssb[:], S0[:], srec[:, :1], None, op0=OP.mult)

    TS2 = psum.tile([2 * NGT, VOC], F32, tag="mmB")
    nc.tensor.matmul(TS2[:], histT, Ssb[:], start=True, stop=True)

    lens = work.tile([2 * NGT, 1], F32, tag="lens")
    nc.vector.tensor_reduce(lens[:], TS2[:], axis=mybir.AxisListType.X, op=OP.add)
    m01 = work.tile([2 * NGT, 1], F32, tag="m01")
    nc.vector.tensor_scalar(m01[:], lens[:], 0.5, None, op0=OP.is_ge)
    m100 = work.tile([2 * NGT, 1], F32, tag="m100")
    nc.vector.tensor_scalar(m100[:], m01[:], -100.0, 100.0, op0=OP.mult, op1=OP.add)
    lenc = work.tile([2 * NGT, 1], F32, tag="lenc")
    nc.vector.tensor_scalar(lenc[:], lens[:], 1.0, None, op0=OP.max)
    rlen = work.tile([2 * NGT, 1], F32, tag="rlen")
    nc.vector.reciprocal(rlen[:], lenc[:])
    ta = work.tile([2 * NGT, VOC], F32, tag="ta")
    nc.vector.tensor_scalar(ta[:], TS2[:], rlen[:, :1], None, op0=OP.mult)
    asum = work.tile([2 * NGT, 1], F32, tag="asum")
    tam = work.tile([2 * NGT, VOC], F32, tag="tam")
    nc.vector.tensor_scalar(tam[:], ta[:], 1e-6, 0.0, op0=OP.max, op1=OP.add,
                            accum_out=asum[:])
    ras = work.tile([2 * NGT, 1], F32, tag="ras")
    nc.vector.reciprocal(ras[:], asum[:])
    tgs = work.tile([2 * NGT, VOC], F32, tag="tgs")
    nc.vector.tensor_scalar(tgs[:], tam[:], ras[:, :1], None, op0=OP.mult)
    tgsn = work.tile([2 * NGT, VOC], BF16, tag="tgsn")
    nc.vector.tensor_scalar(tgsn[:], tgs[:], -1.0, None, op0=OP.mult)
    trp = psum.tile([VOC, 2 * NGT], BF16, tag="mmB")
    nc.tensor.transpose(trp[:], tgsn[:], ident[:64, :64])
    ntgsT = work.tile([VOC, 2 * NGT], BF16, tag="ntgsT")
    nc.vector.tensor_copy(ntgsT[:], trp[:])

    # ---------------- focal class cost (x16, matches product scale) ------
    # computed early: it joins the cdist PSUM accumulation as a
    # rank-1 broadcast matmul (ones x class-row)
    eu = work.tile([NQ, 50], F32, tag="eu")
    nc.scalar.activation(eu[:], pls, AF.Exp, scale=-1.0)
    dr = work.tile([NQ, 50], F32, tag="dr")
    nc.vector.tensor_scalar(dr[:], eu[:], 1.0, None, op0=OP.add)
    nc.vector.reciprocal(dr[:], dr[:])
    s2 = work.tile([NQ, 2], F32, tag="s2")
    nc.vector.tensor_reduce(s2[:], dr[:].rearrange("p (i l) -> p i l", i=2),
                            axis=mybir.AxisListType.X, op=OP.add)
    beps = work.tile([NQ, 1], F32, tag="beps")
    nc.vector.memset(beps[:], 1e-8)
    b1eps = work.tile([NQ, 1], F32, tag="b1eps")
    nc.vector.memset(b1eps[:], 1.0 + 1e-8)
    l1 = work.tile([NQ, 2], F32, tag="l1")
    nc.scalar.activation(l1[:], s2[:], AF.Ln, bias=beps[:], scale=1.0 / NPTS)
    l2 = work.tile([NQ, 2], F32, tag="l2")
    nc.scalar.activation(l2[:], s2[:], AF.Ln, bias=b1eps[:], scale=-1.0 / NPTS)
    pm = work.tile([NQ, 2], F32, tag="pm")
    nc.vector.tensor_scalar(pm[:], s2[:], 1.0 / NPTS, None, op0=OP.mult)
    q1 = work.tile([NQ, 2], F32, tag="q1")
    nc.vector.tensor_scalar(q1[:], pm[:], -1.0, 1.0, op0=OP.mult, op1=OP.add)
    q1s = work.tile([NQ, 2], F32, tag="q1s")
    nc.vector.tensor_tensor(q1s[:], q1[:], q1[:], op=OP.mult)
    pms = work.tile([NQ, 2], F32, tag="pms")
    nc.vector.tensor_tensor(pms[:], pm[:], pm[:], op=OP.mult)
    tA = work.tile([NQ, 2], F32, tag="tA")
    nc.vector.tensor_tensor(tA[:], q1s[:], l1[:], op=OP.mult)
    tB = work.tile([NQ, 2], F32, tag="tB")
    nc.vector.tensor_tensor(tB[:], pms[:], l2[:], op=OP.mult)
    tAs = work.tile([NQ, 2], F32, tag="tAs")
    nc.vector.tensor_scalar(tAs[:], tA[:], -0.25 * 16.0, None, op0=OP.mult)
    ccf = work.tile([NQ, 2], F32, tag="ccf")
    nc.vector.tensor_scalar(ccf[:], tB[:], 0.75 * 16.0, None, op0=OP.mult)
    ccb = work.tile([NQ, 2], BF16, tag="ccb")
    nc.vector.tensor_tensor(ccb[:], ccf[:], tAs[:], op=OP.add)
    ccT = psum.tile([1, NQC], BF16, tag="ccT")
    nc.tensor.transpose(ccT[:, :NQ], ccb[:, 0:1], ident[:NQ, :NQ])
    nc.tensor.transpose(ccT[:, NQ:], ccb[:, 1:2], ident[:NQ, :NQ])
    ccrow = work.tile([1, NQC], BF16, tag="ccrow")
    nc.vector.tensor_copy(ccrow[:], ccT[:])

    # ---------------- cdist rank-R contraction ---------------------------
    PCs = [psum.tile([128, NQC], F32, tag=f"pc{j}", name=f"pc{j}")
           for j in range(4)]

    def cdist_f16(cis):
        for ci in cis:
            rows = CH16[ci]
            for j in range(4):
                nc.tensor.matmul(
                    PCs[j][:],
                    ct16[0:rows, T * ci + 128 * j : T * ci + 128 * j + 128],
                    cq16[0:rows, NQC * ci : NQC * (ci + 1)],
                    start=(ci == 0), stop=False,
                )

    def cdist_fp8_pair(p):
        co = 2 * p
        lt2 = ct8[:, T * co : T * (co + 2)].rearrange("p (two r) -> p two r",
                                                      two=2)
        rq2 = cq8[:, NQC * co : NQC * (co + 2)].rearrange(
            "p (two r) -> p two r", two=2)
        for j in range(4):
            nc.tensor.matmul(
                PCs[j][:],
                lt2[:, :, 128 * j : 128 * j + 128],
                rq2[:],
                start=False, stop=False,
                perf_mode=mybir.MatmulPerfMode.DoubleRow,
            )

    cdist_f16(range(len(CH16)))
    cdist_fp8_pair(0)

    # ---------------- pred text softmax-mean (per slice) -----------------
    ex = work.tile([125, 40 * 97], BF16, tag="ex")
    sums = work.tile([125, 40], BF16, tag="sums")
    rinv = work.tile([125, 40], BF16, tag="rinv")
    selw = work.tile([125, 200], BF16, tag="selw")
    PAT = psum.tile([VOC, NQC], F32, tag="mmA")

    def slice_pre(g0, g1, eng):
        c0, c1 = 97 * g0, 97 * g1
        nc.scalar.activation(ex[:, c0:c1], ptl[:, c0:c1], AF.Exp)
        with nc.allow_low_precision(reason="softmax denom in f16 is plenty"):
            nc.vector.tensor_reduce(
                sums[:, g0:g1],
                ex[:, c0:c1].rearrange("p (g c) -> p g c", g=g1 - g0),
                axis=mybir.AxisListType.X, op=OP.add)
        with nc.allow_low_precision(reason="1/Z in f16: 5e-4 rel on probs"):
            nc.vector.reciprocal(rinv[:, g0:g1], sums[:, g0:g1])
        nc.vector.tensor_tensor(
            selw[:, 5 * g0 : 5 * g1].rearrange("p (g m) -> p g m", g=g1 - g0),
            sel04.rearrange("p (a m) -> p a m", a=1).to_broadcast([125, g1 - g0, 5]),
            rinv[:, g0:g1].rearrange("p (g a) -> p g a", a=1).to_broadcast(
                [125, g1 - g0, 5]),
            op=OP.mult)

    def slice_pat(g0, g1):
        for g in range(g0, g1):
            nc.tensor.matmul(PAT[:, 5 * g : 5 * g + 5],
                             ex[:, 97 * g : 97 * g + VOC],
                             selw[:, 5 * g : 5 * g + 5], start=True, stop=True)

    engs = [nc.vector] * 6
    for k, (g0, g1) in enumerate(GQ[:2]):
        slice_pre(g0, g1, engs[k])
    for g0, g1 in GQ[:2]:
        slice_pat(g0, g1)
    for p in range(1, len(CH8) // 2):
        cdist_fp8_pair(p)
    for k, (g0, g1) in enumerate(GQ[2:]):
        slice_pre(g0, g1, engs[k + 2])
    for g0, g1 in GQ[2:4]:
        slice_pat(g0, g1)
    for g0, g1 in GQ[4:]:
        slice_pat(g0, g1)

    # ---------------- Ln phase (no further table switches) ---------------
    ltg = work.tile([2 * NGT, VOC], F32, tag="ltg")
    nc.scalar.activation(ltg[:], tgs[:], AF.Ln)
    prod = work.tile([2 * NGT, VOC], F32, tag="prod")
    nc.gpsimd.tensor_tensor(prod[:], tgs[:], ltg[:], op=OP.mult)
    ne = work.tile([2 * NGT, 1], F32, tag="ne")
    nc.vector.tensor_reduce(ne[:], prod[:], axis=mybir.AxisListType.X, op=OP.add)

    # pred_avg >= ~4e-3 with VOC=96 softmaxes, so the reference's 1e-6
    # clamp never binds — Ln reads the PAT PSUM directly
    lgp = work.tile([VOC, NQC], BF16, tag="lgp")
    nc.scalar.activation(lgp[:], PAT[:], AF.Ln)

    KL = psum.tile([2 * NGT, NQ], F32, tag="mmB")
    for img in range(NI):
        nc.tensor.matmul(KL[NGT * img : NGT * (img + 1), :],
                         ntgsT[:, NGT * img : NGT * (img + 1)],
                         lgp[:, NQ * img : NQ * (img + 1)], start=True, stop=True)
    tx0 = work.tile([2 * NGT, NQ], F32, tag="tx0")
    nc.vector.tensor_scalar(tx0[:], KL[:], ne[:, :1], 0.0, op0=OP.add, op1=OP.max)
    tx1 = work.tile([2 * NGT, NQ], F16, tag="tx1")
    nc.vector.tensor_scalar(tx1[:], tx0[:], m01[:, :1], m100[:, :1],
                            op0=OP.mult, op1=OP.add)
    nc.scalar.dma_start(t_txt.ap(), tx1[:])
    # class cost joins the accumulation: PCs[j] += ones(128t) x ccrow(200q);
    # each t-group then copies out and streams to HBM immediately
    outsb = work.tile([128, 4 * NQC], F16, tag="outsb")
    for j in range(4):
        nc.tensor.matmul(PCs[j][:], onesr, ccrow[:], start=False, stop=True)
        sl = outsb[:, NQC * j : NQC * (j + 1)]
        if j == 0:
            nc.vector.tensor_copy(sl, PCs[j][:])
        else:
            nc.scalar.copy(sl, PCs[j][:])
        nc.sync.dma_start(t_out.ap()[:, NQC * j : NQC * (j + 1)], sl)


def _get_nc():
    if "nc" not in _CACHE:
        _CACHE["nc"] = _build_program()
    return _CACHE["nc"]


def _install_ntff_hook():
    """Provide antenv.axon_hooks (absent in this image) so that
    run_bass_kernel_spmd(trace=True) can capture NTFF profiles via the
    axon PJRT .so ctypes interface."""
    import types
    try:
        from antenv.axon_hooks import get_axon_ntff_profile_hook  # noqa
        return
    except ImportError:
        pass
    sys.path.insert(0, "/root/.axon_site")
    from trn_agent_boot.trn_boot import _ntff_profile_via_ctypes
    hook = _ntff_profile_via_ctypes("/opt/axon/libaxon_pjrt.so")
    mod = types.ModuleType("antenv.axon_hooks")
    mod._hook = hook
    mod.get_axon_ntff_profile_hook = lambda: mod._hook
    mod.set_axon_ntff_profile_hook = lambda h: setattr(mod, "_hook", h)
    import antenv
    antenv.axon_hooks = mod
    sys.modules["antenv.axon_hooks"] = mod


def _prep_core(pred_logits, pred_ctrl, pred_text, target_texts, c, Fb,
               shared, shared_b16):
    b0 = NI * c
    # pred text logits -> [125=(q5,pt), (g,c)] fp8
    x = pred_text[b0 : b0 + NI].reshape(NQC // 5, 5, NPTS, VOC + 1)
    ptl = np.ascontiguousarray(
        x.transpose(1, 2, 0, 3).reshape(125, 40 * 97)).astype(NPFP8)
    # query-side factor rows, appended after the shared target-side blocks
    qc = pred_ctrl[b0 : b0 + NI].reshape(NQC, D)
    cq16, cq8 = _factor_blocks(qc, Fb, NQC)
    f16b = np.concatenate([shared["ct16"], cq16], axis=1)
    fp8b = np.concatenate([shared["ct8"], cq8], axis=1)
    # bf16 consts: cent | ident | histT | pls | sel04 | ones
    b16c = shared_b16.copy()
    pl = pred_logits[b0 : b0 + NI].reshape(NI, NQ, NPTS).transpose(1, 0, 2)
    b16c[:100, 452:502] = pl.reshape(NQ, 50).astype(NPBF16)
    texts = target_texts[b0 : b0 + NI].reshape(2 * NGT, MAXLEN)
    hist = (texts[:, :, None] == np.arange(VOC)[None, None, :]).sum(axis=1)
    b16c[:VOC, 388:452] = hist.T.astype(NPBF16)
    return {"ptl": ptl, "f16b": f16b, "fp8b": fp8b, "b16c": b16c}


def kernel(pred_logits, pred_ctrl_points, pred_text_logits, tgt_ctrl_points,
           target_texts, centroids):
    pred_logits = np.asarray(pred_logits, np.float32)
    pred_ctrl = np.asarray(pred_ctrl_points, np.float32)
    pred_text = np.asarray(pred_text_logits, np.float32)
    tgt_ctrl = np.asarray(tgt_ctrl_points, np.float32)
    target_texts_np = np.asarray(target_texts, np.int32)
    centroids_np = np.asarray(centroids, np.float32)

    if "basis" not in _CACHE:
        _CACHE["basis"] = _basis()
    Fb, Gb = _CACHE["basis"]

    ct16, ct8 = _factor_blocks(tgt_ctrl.reshape(T, D), Gb, T)
    shared = {"ct16": ct16, "ct8": ct8}

    shared_b16 = np.zeros((128, 635), NPBF16)
    shared_b16[0, 507:635] = 1.0
    centT = centroids_np.T                             # [300, 96]
    for kk, rows in enumerate((128, 128, 44)):
        shared_b16[:rows, 96 * kk : 96 * (kk + 1)] = \
            centT[kk * 128 : kk * 128 + rows, :].astype(NPBF16)
    shared_b16[:100, 288:388] = np.eye(100, dtype=NPBF16)
    s04 = np.zeros((125, 5), np.float32)
    for m in range(5):
        s04[m * 25 : (m + 1) * 25, m] = 1.0 / NPTS
    shared_b16[:125, 502:507] = s04.astype(NPBF16)

    in_maps = [
        _prep_core(pred_logits, pred_ctrl, pred_text, target_texts_np, c, Fb,
                   shared, shared_b16)
        for c in range(NCORES)
    ]

    nc = _get_nc()
    import os
    trace = bool(os.environ.get("KERNEL_TRACE"))
    if trace:
        _install_ntff_hook()
    try:
        res = bass_utils.run_bass_kernel_spmd(
            nc, in_maps, core_ids=list(range(NCORES)), trace=trace,
            trace_cores=list(range(NCORES)) if trace else None)
    except ModuleNotFoundError:
        res = bass_utils.run_bass_kernel_spmd(
            nc, in_maps, core_ids=list(range(NCORES)), trace=False)
    if trace and res.exec_time_ns is not None:
        _CACHE["exec_time_ns"] = res.exec_time_ns
        _CACHE["mean_exec_time_ns"] = res.mean_exec_time_ns

    # host assembly: [128, 4*200]/16 -> [200q, 512t] per core + text block
    C = np.empty((BS, NQ, T), np.float32)
    for c in range(NCORES):
        outc = res.results[c]["outC"].astype(np.float32) * (1.0 / 16.0)
        outt = res.results[c]["outT"].astype(np.float32)   # [64, 100]
        full = np.ascontiguousarray(
            outc.reshape(128, 4, NQC).transpose(1, 0, 2).reshape(T, NQC))
        for img in range(NI):
            b = NI * c + img
            blk = full[:, NQ * img : NQ * (img + 1)].T.copy()   # [100, 512]
            blk[:, b * NGT : (b + 1) * NGT] += \
                outt[NGT * img : NGT * (img + 1), :].T
            C[b] = blk
    return C


# revision 73
# speedup vs baseline: 1.1625x; 1.1625x over previous
"""Trainium2 Bass kernel for CtrlPointHungarianMatcher cost matrix.

Computes C[b,q, b'*NGT+g] = class_cost[b,q] + L1_cdist + blockdiag(text_KL).

Sharding: data-parallel over batch; core c handles images (2c, 2c+1) =
200 queries x all 512 targets.

Design notes (final):
- The L1 cdist is a rank-14 bilinear form: |x-y| on [0,1]^2 is
  approximated by sum_r f_r(x) g_r(y) (SVD of the kernel on a 256-pt
  grid; max abs err ~0.11 on 50-dim sums, ~1.3% of the smallest C).
  Host evaluates the factor tables at input coords (input encoding,
  like the baseline's char histogram); the device contracts the
  [701-row] factors into 4 [128t, 200q] PSUM tiles.  Ranks 0-3 ride
  in f16 (normal matmuls); ranks 4-13 in float8_e4m3 run PAIRWISE in
  DoubleRow perf mode (2 contraction chunks per instruction).  All
  sides are scaled x4 (uniform x16 product scale, divided out on the
  host), which keeps every value in the fp8 normal range.
- pred_text_logits are shipped as float8_e3m4 (error on log-probs
  <0.05, ~0.5% of C) — halves the dominant input DMA vs bf16.
- All activations are Exp/Ln/Copy; a combined exp+ln table
  (natural_log_exp_and_others) is pre-loaded once at program start so
  the ACT engine never reloads tables (an earlier trace showed
  6 x 1.28us of exp<->ln table thrash).  Sigmoid for the focal cost
  is exp+reciprocal to stay inside that one table.
- The focal class cost joins the cdist PSUM accumulation as a rank-1
  broadcast matmul (ones[1,128] x class-row[1,200]), computed early so
  it never stalls the accumulation's stop.
- Inputs stream on three parallel DMA queues (SP: consts+ptl slices,
  ACT: f16 factors, Pool: fp8 factors); per-queue throughput is only
  ~160GB/s so queue parallelism matters.  ptl lands in 3 slices and
  is exp/softmax'd in 5 slices of 8 groups so the softmax pipeline
  overlaps the stream; each [128t, 200q] output block DMAs out as
  soon as its PSUM->SBUF copy lands.
Host un-permutes [128,4*200] -> [200,512]/16 per core and adds the
text block diagonally (core-dependent column offset; SPMD program is
shared).
"""

import sys

sys.path.insert(0, "/opt/trn_rl_repo")

from contextlib import ExitStack

import ml_dtypes
import numpy as np

from concourse import bacc, bass, mybir, tile
from concourse import bass_utils

BF16 = mybir.dt.bfloat16
F32 = mybir.dt.float32
F16 = mybir.dt.float16
FP8 = mybir.dt.float8e3       # e3m4: pred-text logits (best <1 precision)
FP8W = mybir.dt.float8e4      # e4m3: cdist tail-rank factors (DoubleRow-able)
AF = mybir.ActivationFunctionType
OP = mybir.AluOpType

NPBF16 = ml_dtypes.bfloat16
NPFP8 = ml_dtypes.float8_e3m4
NPFP8W = ml_dtypes.float8_e4m3fn

BS, NQ, NPTS, VOC, MAXLEN, NGT, EDIM = 16, 100, 25, 96, 25, 32, 300
NCORES = 8
NI = BS // NCORES          # images per core = 2
T = BS * NGT               # 512 targets
D = NPTS * 2               # 50 coord dims
NQC = NI * NQ              # 200 queries per core
INV_SQRT_E = float(1.0 / np.sqrt(np.float32(EDIM)))

# rank-R bilinear factorization of |x-y|; ranks [0,NH) f16, [NH,R) fp8e4.
# fp8 chunks run pairwise in DoubleRow mode (2 contraction rows/PE row),
# so the fp8 chunk count is padded to even.
RNK, NH = 14, 4
GRID = 256
N16 = D * NH               # 200 f16 rows
N8 = D * (RNK - NH)        # 700 fp8 rows
CH16 = [min(128, N16 - 128 * c) for c in range((N16 + 127) // 128)]  # [128, 72]
NCH8 = -2 * ((N8 + 127) // 128 // -2)                                # 6 (even)
CH8 = [min(128, max(0, N8 - 128 * c)) for c in range(NCH8)]
SCL = 4.0                  # per-side scale; product scale 16, host divides

# ptl DMA'd in 3 slices; exp/sums processed in 5 slices of 8 groups
# (40 softmax groups of 97 cols)
GDMA = [(0, 16), (16, 32), (32, 40)]
GQ = [(0, 8), (8, 16), (16, 24), (24, 32), (32, 38), (38, 40)]



_CACHE = {}


def _basis():
    x = (np.arange(GRID, dtype=np.float64) + 0.5) / GRID
    A = np.abs(x[:, None] - x[None, :])
    U, s, Vt = np.linalg.svd(A)
    Fb = (U[:, :RNK] * np.sqrt(s[:RNK])).astype(np.float32)
    Gb = (Vt[:RNK].T * np.sqrt(s[:RNK])).astype(np.float32)
    return Fb, Gb


def _ev(P, pts):
    """Linear interp of basis table P [GRID, R] at pts [...] -> [..., R]."""
    idx = np.clip(pts.astype(np.float64) * GRID - 0.5, 0, GRID - 1 - 1e-9)
    i0 = np.floor(idx).astype(np.int32)
    fr = (idx - i0)[..., None].astype(np.float32)
    i1 = np.minimum(i0 + 1, GRID - 1)
    return P[i0] * (1 - fr) + P[i1] * fr


def _chunk(rows, nch, width, dtype):
    """[n_used, width] -> [128, nch*width] chunk-major layout."""
    arr = np.zeros((nch * 128, width), dtype)
    arr[: rows.shape[0]] = rows
    return np.ascontiguousarray(
        arr.reshape(nch, 128, width).transpose(1, 0, 2).reshape(128, nch * width))


def _factor_blocks(coords, P, width):
    """coords [width, 50] -> (f16 block [128, len(CH16)*width],
    fp8e4 block [128, len(CH8)*width]), scaled by SCL."""
    fv = _ev(P, coords) * SCL                       # [width, 50, R]
    hi = fv[..., :NH].transpose(1, 2, 0).reshape(N16, width)
    lo = fv[..., NH:].transpose(1, 2, 0).reshape(N8, width)
    return (_chunk(hi.astype(np.float16), len(CH16), width, np.float16),
            _chunk(lo.astype(NPFP8W), len(CH8), width, NPFP8W))


def _build_program():
    nc = bacc.Bacc("TRN2", debug=False, num_devices=NCORES)

    t_ptl = nc.dram_tensor("ptl", [125, 40 * 97], FP8, kind="ExternalInput")
    t_f16b = nc.dram_tensor("f16b", [128, len(CH16) * (T + NQC)], F16,
                            kind="ExternalInput")
    t_fp8b = nc.dram_tensor("fp8b", [128, len(CH8) * (T + NQC)], FP8W,
                            kind="ExternalInput")
    t_b16 = nc.dram_tensor("b16c", [128, 635], BF16, kind="ExternalInput")

    t_out = nc.dram_tensor("outC", [128, 4 * NQC], F16, kind="ExternalOutput")
    t_txt = nc.dram_tensor("outT", [2 * NGT, NQ], F16, kind="ExternalOutput")

    with tile.TileContext(nc) as tc:
        with ExitStack() as ctx:
            _body(ctx, tc, t_ptl, t_f16b, t_fp8b, t_b16, t_out, t_txt)
    nc.compile()
    return nc


def _act_table_id(arch):
    from concourse.hw_specs import get_activation_tables
    tables = get_activation_tables(arch)
    for i, (name, funcs) in enumerate(tables.items()):
        if name == "natural_log_exp_and_others":
            return i
    return None


def _body(ctx, tc, t_ptl, t_f16b, t_fp8b, t_b16, t_out, t_txt):
    nc = tc.nc

    const = ctx.enter_context(tc.tile_pool(name="const", bufs=1))
    work = ctx.enter_context(tc.tile_pool(name="work", bufs=1))
    psum = ctx.enter_context(tc.tile_pool(name="psum", bufs=1, space="PSUM"))

    # pre-load the combined exp+ln table so no reloads happen mid-kernel
    tid = _act_table_id(nc.m.arch)
    if tid is not None:
        ld = mybir.InstLoadActFuncSet(
            name=nc.get_next_instruction_name(), act_func_set_id=tid)
        nc.scalar.add_instruction(ld)



    # ---------------- input DMAs: three hardware queues in parallel ------
    # (each queue streams ~100-160GB/s; queues run concurrently)
    bb = const.tile([128, 635], BF16, tag="bb")
    nc.sync.dma_start(bb[:], t_b16.ap())
    ptl = work.tile([125, 40 * 97], FP8, tag="ptl")
    for g0, g1 in GDMA:
        nc.sync.dma_start(ptl[:, 97 * g0 : 97 * g1],
                          t_ptl.ap()[:, 97 * g0 : 97 * g1])
    f16b = const.tile([128, len(CH16) * (T + NQC)], F16, tag="f16b")
    nc.scalar.dma_start(f16b[:], t_f16b.ap())
    fp8b = const.tile([128, len(CH8) * (T + NQC)], FP8W, tag="fp8b")
    nc.gpsimd.dma_start(fp8b[:], t_fp8b.ap())
    ct16 = f16b[:, : len(CH16) * T]
    cq16 = f16b[:, len(CH16) * T :]
    ct8 = fp8b[:, : len(CH8) * T]
    cq8 = fp8b[:, len(CH8) * T :]

    cent = bb[:, 0:288]
    ident = bb[:, 288:388]
    histT = bb[:96, 388:452]
    pls = bb[:100, 452:502]
    sel04 = bb[:125, 502:507]
    onesr = bb[0:1, 507:635]

    # ---------------- target text distributions -------------------------
    G = psum.tile([VOC, VOC], F32, tag="mmA")
    for kk in range(3):
        cch = cent[:, 96 * kk : 96 * (kk + 1)]
        nc.tensor.matmul(G[:], cch, cch, start=(kk == 0), stop=(kk == 2))


    gmaxn = work.tile([VOC, 1], F32, tag="gmaxn")
    nc.vector.tensor_reduce(gmaxn[:], G[:], axis=mybir.AxisListType.X,
                            op=OP.max, negate=True)
    gbias = work.tile([VOC, 1], F32, tag="gbias")
    nc.vector.tensor_scalar(gbias[:], gmaxn[:], INV_SQRT_E, None, op0=OP.mult)
    S0 = work.tile([VOC, VOC], F32, tag="S0")
    ssum = work.tile([VOC, 1], F32, tag="ssum")
    nc.scalar.activation(S0[:], G[:], AF.Exp, bias=gbias[:], scale=INV_SQRT_E,
                         accum_out=ssum[:])
    srec = work.tile([VOC, 1], F32, tag="srec")
    nc.vector.reciprocal(srec[:], ssum[:])
    Ssb = work.tile([VOC, VOC], BF16, tag="Ssb")
    nc.vector.tensor_scalar(Ssb[:], S0[:], srec[:, :1], None, op0=OP.mult)

    TS2 = psum.tile([2 * NGT, VOC], F32, tag="mmB")
    nc.tensor.matmul(TS2[:], histT, Ssb[:], start=True, stop=True)

    lens = work.tile([2 * NGT, 1], F32, tag="lens")
    nc.vector.tensor_reduce(lens[:], TS2[:], axis=mybir.AxisListType.X, op=OP.add)
    m01 = work.tile([2 * NGT, 1], F32, tag="m01")
    nc.vector.tensor_scalar(m01[:], lens[:], 0.5, None, op0=OP.is_ge)
    m100 = work.tile([2 * NGT, 1], F32, tag="m100")
    nc.vector.tensor_scalar(m100[:], m01[:], -100.0, 100.0, op0=OP.mult, op1=OP.add)
    lenc = work.tile([2 * NGT, 1], F32, tag="lenc")
    nc.vector.tensor_scalar(lenc[:], lens[:], 1.0, None, op0=OP.max)
    rlen = work.tile([2 * NGT, 1], F32, tag="rlen")
    nc.vector.reciprocal(rlen[:], lenc[:])
    ta = work.tile([2 * NGT, VOC], F32, tag="ta")
    nc.vector.tensor_scalar(ta[:], TS2[:], rlen[:, :1], None, op0=OP.mult)
    asum = work.tile([2 * NGT, 1], F32, tag="asum")
    tam = work.tile([2 * NGT, VOC], F32, tag="tam")
    nc.vector.tensor_scalar(tam[:], ta[:], 1e-6, 0.0, op0=OP.max, op1=OP.add,
                            accum_out=asum[:])
    ras = work.tile([2 * NGT, 1], F32, tag="ras")
    nc.vector.reciprocal(ras[:], asum[:])
    tgs = work.tile([2 * NGT, VOC], F32, tag="tgs")
    nc.vector.tensor_scalar(tgs[:], tam[:], ras[:, :1], None, op0=OP.mult)
    tgsn = work.tile([2 * NGT, VOC], BF16, tag="tgsn")
    nc.vector.tensor_scalar(tgsn[:], tgs[:], -1.0, None, op0=OP.mult)
    trp = psum.tile([VOC, 2 * NGT], BF16, tag="mmB")
    nc.tensor.transpose(trp[:], tgsn[:], ident[:64, :64])
    ntgsT = work.tile([VOC, 2 * NGT], BF16, tag="ntgsT")
    nc.vector.tensor_copy(ntgsT[:], trp[:])

    # ---------------- focal class cost (x16, matches product scale) ------
    # computed early: it joins the cdist PSUM accumulation as a
    # rank-1 broadcast matmul (ones x class-row)
    eu = work.tile([NQ, 50], F32, tag="eu")
    nc.scalar.activation(eu[:], pls, AF.Exp, scale=-1.0)
    dr = work.tile([NQ, 50], F32, tag="dr")
    nc.vector.tensor_scalar(dr[:], eu[:], 1.0, None, op0=OP.add)
    nc.vector.reciprocal(dr[:], dr[:])
    s2 = work.tile([NQ, 2], F32, tag="s2")
    nc.vector.tensor_reduce(s2[:], dr[:].rearrange("p (i l) -> p i l", i=2),
                            axis=mybir.AxisListType.X, op=OP.add)
    beps = work.tile([NQ, 1], F32, tag="beps")
    nc.vector.memset(beps[:], 1e-8)
    b1eps = work.tile([NQ, 1], F32, tag="b1eps")
    nc.vector.memset(b1eps[:], 1.0 + 1e-8)
    l1 = work.tile([NQ, 2], F32, tag="l1")
    nc.scalar.activation(l1[:], s2[:], AF.Ln, bias=beps[:], scale=1.0 / NPTS)
    l2 = work.tile([NQ, 2], F32, tag="l2")
    nc.scalar.activation(l2[:], s2[:], AF.Ln, bias=b1eps[:], scale=-1.0 / NPTS)
    pm = work.tile([NQ, 2], F32, tag="pm")
    nc.vector.tensor_scalar(pm[:], s2[:], 1.0 / NPTS, None, op0=OP.mult)
    q1 = work.tile([NQ, 2], F32, tag="q1")
    nc.vector.tensor_scalar(q1[:], pm[:], -1.0, 1.0, op0=OP.mult, op1=OP.add)
    q1s = work.tile([NQ, 2], F32, tag="q1s")
    nc.vector.tensor_tensor(q1s[:], q1[:], q1[:], op=OP.mult)
    pms = work.tile([NQ, 2], F32, tag="pms")
    nc.vector.tensor_tensor(pms[:], pm[:], pm[:], op=OP.mult)
    tA = work.tile([NQ, 2], F32, tag="tA")
    nc.vector.tensor_tensor(tA[:], q1s[:], l1[:], op=OP.mult)
    tB = work.tile([NQ, 2], F32, tag="tB")
    nc.vector.tensor_tensor(tB[:], pms[:], l2[:], op=OP.mult)
    tAs = work.tile([NQ, 2], F32, tag="tAs")
    nc.vector.tensor_scalar(tAs[:], tA[:], -0.25 * 16.0, None, op0=OP.mult)
    ccf = work.tile([NQ, 2], F32, tag="ccf")
    nc.vector.tensor_scalar(ccf[:], tB[:], 0.75 * 16.0, None, op0=OP.mult)
    ccb = work.tile([NQ, 2], BF16, tag="ccb")
    nc.vector.tensor_tensor(ccb[:], ccf[:], tAs[:], op=OP.add)
    ccT = psum.tile([1, NQC], BF16, tag="ccT")
    nc.tensor.transpose(ccT[:, :NQ], ccb[:, 0:1], ident[:NQ, :NQ])
    nc.tensor.transpose(ccT[:, NQ:], ccb[:, 1:2], ident[:NQ, :NQ])
    ccrow = work.tile([1, NQC], BF16, tag="ccrow")
    nc.vector.tensor_copy(ccrow[:], ccT[:])

    # ---------------- cdist rank-R contraction ---------------------------
    PCs = [psum.tile([128, NQC], F32, tag=f"pc{j}", name=f"pc{j}")
           for j in range(4)]

    def cdist_f16(cis):
        for ci in cis:
            rows = CH16[ci]
            for j in range(4):
                nc.tensor.matmul(
                    PCs[j][:],
                    ct16[0:rows, T * ci + 128 * j : T * ci + 128 * j + 128],
                    cq16[0:rows, NQC * ci : NQC * (ci + 1)],
                    start=(ci == 0), stop=False,
                )

    def cdist_fp8_pair(p):
        co = 2 * p
        lt2 = ct8[:, T * co : T * (co + 2)].rearrange("p (two r) -> p two r",
                                                      two=2)
        rq2 = cq8[:, NQC * co : NQC * (co + 2)].rearrange(
            "p (two r) -> p two r", two=2)
        for j in range(4):
            nc.tensor.matmul(
                PCs[j][:],
                lt2[:, :, 128 * j : 128 * j + 128],
                rq2[:],
                start=False, stop=False,
                perf_mode=mybir.MatmulPerfMode.DoubleRow,
            )

    cdist_f16(range(len(CH16)))
    cdist_fp8_pair(0)

    # ---------------- pred text softmax-mean (per slice) -----------------
    ex = work.tile([125, 40 * 97], BF16, tag="ex")
    sums = work.tile([125, 40], BF16, tag="sums")
    rinv = work.tile([125, 40], BF16, tag="rinv")
    selw = work.tile([125, 200], BF16, tag="selw")
    PAT = psum.tile([VOC, NQC], F32, tag="mmA")

    def slice_pre(g0, g1, eng):
        c0, c1 = 97 * g0, 97 * g1
        if g1 - g0 <= 2:
            # tail slice: fold the group-sum into the exp via the ACT
            # accumulator (sums all 97 cols = Z), skipping the DVE hop
            with nc.allow_low_precision(reason="softmax denom in bf16"):
                for g in range(g0, g1):
                    nc.scalar.activation(ex[:, 97 * g : 97 * (g + 1)],
                                         ptl[:, 97 * g : 97 * (g + 1)],
                                         AF.Exp, accum_out=sums[:, g : g + 1])
        else:
            nc.scalar.activation(ex[:, c0:c1], ptl[:, c0:c1], AF.Exp)
            with nc.allow_low_precision(reason="softmax denom in f16"):
                nc.vector.tensor_reduce(
                    sums[:, g0:g1],
                    ex[:, c0:c1].rearrange("p (g c) -> p g c", g=g1 - g0),
                    axis=mybir.AxisListType.X, op=OP.add)
        with nc.allow_low_precision(reason="1/Z in f16: 5e-4 rel on probs"):
            nc.vector.reciprocal(rinv[:, g0:g1], sums[:, g0:g1])
        nc.vector.tensor_tensor(
            selw[:, 5 * g0 : 5 * g1].rearrange("p (g m) -> p g m", g=g1 - g0),
            sel04.rearrange("p (a m) -> p a m", a=1).to_broadcast([125, g1 - g0, 5]),
            rinv[:, g0:g1].rearrange("p (g a) -> p g a", a=1).to_broadcast(
                [125, g1 - g0, 5]),
            op=OP.mult)

    def slice_pat(g0, g1):
        for g in range(g0, g1):
            nc.tensor.matmul(PAT[:, 5 * g : 5 * g + 5],
                             ex[:, 97 * g : 97 * g + VOC],
                             selw[:, 5 * g : 5 * g + 5], start=True, stop=True)

    engs = [nc.vector] * 6
    for k, (g0, g1) in enumerate(GQ[:2]):
        slice_pre(g0, g1, engs[k])
    for g0, g1 in GQ[:2]:
        slice_pat(g0, g1)
    for p in range(1, len(CH8) // 2):
        cdist_fp8_pair(p)
    for k, (g0, g1) in enumerate(GQ[2:]):
        slice_pre(g0, g1, engs[k + 2])
    for g0, g1 in GQ[2:4]:
        slice_pat(g0, g1)
    for g0, g1 in GQ[4:]:
        slice_pat(g0, g1)

    # ---------------- Ln phase (no further table switches) ---------------
    ltg = work.tile([2 * NGT, VOC], F32, tag="ltg")
    nc.scalar.activation(ltg[:], tgs[:], AF.Ln)
    prod = work.tile([2 * NGT, VOC], F32, tag="prod")
    nc.gpsimd.tensor_tensor(prod[:], tgs[:], ltg[:], op=OP.mult)
    ne = work.tile([2 * NGT, 1], F32, tag="ne")
    nc.vector.tensor_reduce(ne[:], prod[:], axis=mybir.AxisListType.X, op=OP.add)

    # pred_avg >= ~4e-3 with VOC=96 softmaxes, so the reference's 1e-6
    # clamp never binds — Ln reads the PAT PSUM directly
    lgp = work.tile([VOC, NQC], BF16, tag="lgp")
    nc.scalar.activation(lgp[:], PAT[:], AF.Ln)

    KL = psum.tile([2 * NGT, NQ], F32, tag="mmB")
    for img in range(NI):
        nc.tensor.matmul(KL[NGT * img : NGT * (img + 1), :],
                         ntgsT[:, NGT * img : NGT * (img + 1)],
                         lgp[:, NQ * img : NQ * (img + 1)], start=True, stop=True)
    tx0 = work.tile([2 * NGT, NQ], F32, tag="tx0")
    nc.vector.tensor_scalar(tx0[:], KL[:], ne[:, :1], 0.0, op0=OP.add, op1=OP.max)
    tx1 = work.tile([2 * NGT, NQ], F16, tag="tx1")
    nc.vector.tensor_scalar(tx1[:], tx0[:], m01[:, :1], m100[:, :1],
                            op0=OP.mult, op1=OP.add)
    nc.scalar.dma_start(t_txt.ap(), tx1[:])
    # class cost joins the accumulation: PCs[j] += ones(128t) x ccrow(200q);
    # each t-group then copies out and streams to HBM immediately
    outsb = work.tile([128, 4 * NQC], F16, tag="outsb")
    for j in range(4):
        nc.tensor.matmul(PCs[j][:], onesr, ccrow[:], start=False, stop=True)
        sl = outsb[:, NQC * j : NQC * (j + 1)]
        if j == 0:
            nc.vector.tensor_copy(sl, PCs[j][:])
        else:
            nc.scalar.copy(sl, PCs[j][:])
        nc.sync.dma_start(t_out.ap()[:, NQC * j : NQC * (j + 1)], sl)


def _get_nc():
    if "nc" not in _CACHE:
        _CACHE["nc"] = _build_program()
    return _CACHE["nc"]


def _install_ntff_hook():
    """Provide antenv.axon_hooks (absent in this image) so that
    run_bass_kernel_spmd(trace=True) can capture NTFF profiles via the
    axon PJRT .so ctypes interface."""
    import types
    try:
        from antenv.axon_hooks import get_axon_ntff_profile_hook  # noqa
        return
    except ImportError:
        pass
    sys.path.insert(0, "/root/.axon_site")
    from trn_agent_boot.trn_boot import _ntff_profile_via_ctypes
    hook = _ntff_profile_via_ctypes("/opt/axon/libaxon_pjrt.so")
    mod = types.ModuleType("antenv.axon_hooks")
    mod._hook = hook
    mod.get_axon_ntff_profile_hook = lambda: mod._hook
    mod.set_axon_ntff_profile_hook = lambda h: setattr(mod, "_hook", h)
    import antenv
    antenv.axon_hooks = mod
    sys.modules["antenv.axon_hooks"] = mod


def _prep_core(pred_logits, pred_ctrl, pred_text, target_texts, c, Fb,
               shared, shared_b16):
    b0 = NI * c
    # pred text logits -> [125=(q5,pt), (g,c)] fp8
    x = pred_text[b0 : b0 + NI].reshape(NQC // 5, 5, NPTS, VOC + 1)
    ptl = np.ascontiguousarray(
        x.transpose(1, 2, 0, 3).reshape(125, 40 * 97)).astype(NPFP8)
    # query-side factor rows, appended after the shared target-side blocks
    qc = pred_ctrl[b0 : b0 + NI].reshape(NQC, D)
    cq16, cq8 = _factor_blocks(qc, Fb, NQC)
    f16b = np.concatenate([shared["ct16"], cq16], axis=1)
    fp8b = np.concatenate([shared["ct8"], cq8], axis=1)
    # bf16 consts: cent | ident | histT | pls | sel04 | ones
    b16c = shared_b16.copy()
    pl = pred_logits[b0 : b0 + NI].reshape(NI, NQ, NPTS).transpose(1, 0, 2)
    b16c[:100, 452:502] = pl.reshape(NQ, 50).astype(NPBF16)
    texts = target_texts[b0 : b0 + NI].reshape(2 * NGT, MAXLEN)
    hist = (texts[:, :, None] == np.arange(VOC)[None, None, :]).sum(axis=1)
    b16c[:VOC, 388:452] = hist.T.astype(NPBF16)
    return {"ptl": ptl, "f16b": f16b, "fp8b": fp8b, "b16c": b16c}


def kernel(pred_logits, pred_ctrl_points, pred_text_logits, tgt_ctrl_points,
           target_texts, centroids):
    pred_logits = np.asarray(pred_logits, np.float32)
    pred_ctrl = np.asarray(pred_ctrl_points, np.float32)
    pred_text = np.asarray(pred_text_logits, np.float32)
    tgt_ctrl = np.asarray(tgt_ctrl_points, np.float32)
    target_texts_np = np.asarray(target_texts, np.int32)
    centroids_np = np.asarray(centroids, np.float32)

    if "basis" not in _CACHE:
        _CACHE["basis"] = _basis()
    Fb, Gb = _CACHE["basis"]

    ct16, ct8 = _factor_blocks(tgt_ctrl.reshape(T, D), Gb, T)
    shared = {"ct16": ct16, "ct8": ct8}

    shared_b16 = np.zeros((128, 635), NPBF16)
    shared_b16[0, 507:635] = 1.0
    centT = centroids_np.T                             # [300, 96]
    for kk, rows in enumerate((128, 128, 44)):
        shared_b16[:rows, 96 * kk : 96 * (kk + 1)] = \
            centT[kk * 128 : kk * 128 + rows, :].astype(NPBF16)
    shared_b16[:100, 288:388] = np.eye(100, dtype=NPBF16)
    s04 = np.zeros((125, 5), np.float32)
    for m in range(5):
        s04[m * 25 : (m + 1) * 25, m] = 1.0 / NPTS
    shared_b16[:125, 502:507] = s04.astype(NPBF16)

    in_maps = [
        _prep_core(pred_logits, pred_ctrl, pred_text, target_texts_np, c, Fb,
                   shared, shared_b16)
        for c in range(NCORES)
    ]

    nc = _get_nc()
    import os
    trace = bool(os.environ.get("KERNEL_TRACE"))
    if trace:
        _install_ntff_hook()
    try:
        res = bass_utils.run_bass_kernel_spmd(
            nc, in_maps, core_ids=list(range(NCORES)), trace=trace,
            trace_cores=list(range(NCORES)) if trace else None)
    except ModuleNotFoundError:
        res = bass_utils.run_bass_kernel_spmd(
            nc, in_maps, core_ids=list(range(NCORES)), trace=False)
    if trace and res.exec_time_ns is not None:
        _CACHE["exec_time_ns"] = res.exec_time_ns
        _CACHE["mean_exec_time_ns"] = res.mean_exec_time_ns

    # host assembly: [128, 4*200]/16 -> [200q, 512t] per core + text block
    C = np.empty((BS, NQ, T), np.float32)
    for c in range(NCORES):
        outc = res.results[c]["outC"].astype(np.float32) * (1.0 / 16.0)
        outt = res.results[c]["outT"].astype(np.float32)   # [64, 100]
        full = np.ascontiguousarray(
            outc.reshape(128, 4, NQC).transpose(1, 0, 2).reshape(T, NQC))
        for img in range(NI):
            b = NI * c + img
            blk = full[:, NQ * img : NQ * (img + 1)].T.copy()   # [100, 512]
            blk[:, b * NGT : (b + 1) * NGT] += \
                outt[NGT * img : NGT * (img + 1), :].T
            C[b] = blk
    return C
